# revision 23
# baseline (speedup 1.0000x reference)
"""Paged GQA decode attention (sparse_attention) on 8 TRN2 NeuronCores.

Sharding: tensor-parallel by KV head (8 heads -> 8 cores). Each core gets its
head's slice of the KV pool as combined bf16 rows [K(128)|V(128)] = 512B so a
single dma_gather descriptor per token fetches both K and V at the efficient
512B descriptor size (sub-512B descriptors pay a 2x DMA latency multiplier).

Per core dataflow (program fully specialized at build time on seq_lens meta,
identical across cores):
  gather: one transposed dma_gather per group-half -> plane0 = K^T [d,s]
          (directly usable by QK), plane1 = V^T [d,s].
  QK:     scores^T[s,4] = K^T_slot.T @ q_cols  (bf16, PSUM f32)
  exp:    one ACT Exp per group bank -> phi (bf16) in SBUF
  V^T->V: PE transpose per 128-token slot into bf16 PSUM banks (8 slots per
          bank), evacuated to SBUF by DVE/ACT alternately.
  PV:     o^T[4,128] += phi_slot @ V_slot  (bf16, PSUM f32 accum)
  sums:   ones^T @ phi -> per-slot-group softmax partial sums (one matmul
          per group); host does the final normalization.

Padding: slots are padded per request-half to 128 tokens using a zeroed,
unreferenced pool row. K=0 => score 0 => exp=1 exactly; V=0 contributes
nothing; the host subtracts the exact pad count from each request's softmax
denominator. No masking work on device.
"""

import numpy as np
import ml_dtypes

import concourse.bacc as bacc
import concourse.mybir as mybir
import concourse.tile as tile
from concourse.bass_utils import run_bass_kernel_spmd

B, S, HQ, HKV, D, G = 32, 2048, 32, 8, 128, 4
POOL = B * S
HALF = POOL // 2
SCALE = D ** -0.5
NCORES = 8
GROUPS = 8
RPG = B // GROUPS  # requests per group

BF16 = ml_dtypes.bfloat16

_prog_cache: dict = {}
LAST_RESULT = None  # test.py introspection (exec time etc.)


def _pad32(n):
    return (n + 31) // 32 * 32


def _pad128(n):
    return (n + 127) // 128 * 128


def _sec_pads(secs):
    """Padded section lengths. Sections are padded to full 128-token slots:
    sub-slot (partition-offset) matmul outputs smear the executor's PSUM
    group-check flags across other banks (the check's flat-layout shadow
    aliases t0*row_stride into foreign banks), falsely colliding with open
    PV accumulation chains — so every matmul output must sit at partition
    base 0. Returns (padded lengths, total n)."""
    pads = [_pad128(sec) for sec in secs]
    return pads, sum(pads)


def _cut_pieces(a, b):
    """Cut token range [a, b) (32-aligned, never starting at 96 mod 128)
    into PE tile-position-legal pieces (token_start, len): a piece never
    crosses a 128 slot boundary and starts at offset 0, 32 or 64 (offset 0:
    len<=128, offset 32: len 32, offset 64: len<=64)."""
    out = []
    while a < b:
        t0 = a % 128
        slot_end = a - t0 + 128
        if t0 in (0, 64):
            take = min(b, slot_end) - a
        else:
            assert t0 == 32, t0
            take = 32
        out.append((a, take))
        a += take
    return out


def _layout(meta):
    """meta[g][h][j] = valid token count of request j in half h of group g.

    Sections are padded to 32 tokens (pad idx -> zeroed pool row); each
    group-half is padded to a 128 multiple. Returns per-group: halves
    (n, ioff), slot count, slot->(half, local slot) map, and the piece list
    (gslot, t0, len, owner j or None, piece_id); sm columns are piece-major.
    """
    info = []
    icol = 0  # running column offset into the merged idx tensor
    for g in range(GROUPS):
        halves = []
        pieces = []  # (gslot, t0, len, j_or_None, pid)
        req_pieces = [[] for _ in range(RPG)]
        slot_base = 0
        for h in (0, 1):
            secs = meta[g][h]
            pads, n = _sec_pads(secs)
            pos = 0
            sec_ranges = []
            for j in range(RPG):
                sec_ranges.append((j, pos, pos + pads[j]))
                pos += pads[j]
            if pos < n:
                sec_ranges.append((None, pos, n))  # group-half tail pad
            halves.append(dict(n=n, slots=n // 128, ioff=icol, secs=secs))
            icol += n // 16
            for j, a, b in sec_ranges:
                for (p0, ln) in _cut_pieces(a, b):
                    pid = len(pieces)
                    pieces.append((slot_base + p0 // 128, p0 % 128, ln, j, pid))
                    if j is not None:
                        req_pieces[j].append(pid)
            slot_base += n // 128
        n_lo = halves[0]["slots"]
        nslots = n_lo + halves[1]["slots"]
        slot_map = [(0, i) for i in range(n_lo)] + \
                   [(1, i) for i in range(halves[1]["slots"])]
        info.append(dict(halves=halves, nslots=nslots, slot_map=slot_map,
                         pieces=pieces, req_pieces=req_pieces))
    return info, icol


def _build_program(meta):
    info, idx_w = _layout(meta)
    dt = mybir.dt
    nc = bacc.Bacc(trn_type="TRN2")

    kv_il = nc.dram_tensor("kv_il", [POOL, 256], dt.bfloat16, kind="ExternalInput")
    qT_d = nc.dram_tensor("qT", [128, 128], dt.bfloat16, kind="ExternalInput")
    ident_d = nc.dram_tensor("ident", [128, 128], dt.bfloat16, kind="ExternalInput")
    idx_w = max(1, idx_w)
    idx_d = nc.dram_tensor("idx_all", [128, idx_w], dt.int16, kind="ExternalInput")
    OC = RPG * D  # output cols per group
    o_dram = nc.dram_tensor("o_un", [G, B * D], dt.float32, kind="ExternalOutput")
    s_dram = nc.dram_tensor("sums", [GROUPS, 512], dt.float32, kind="ExternalOutput")

    with tile.TileContext(nc) as tc:
        with (
            tc.tile_pool(name="const", bufs=1) as cpool,
            tc.tile_pool(name="kvt", bufs=4) as kvtp,
            tc.tile_pool(name="vnat", bufs=2) as vnp,
            tc.tile_pool(name="phi", bufs=2) as php,
            tc.tile_pool(name="stg", bufs=2) as stgp,
            tc.tile_pool(name="ps_sc", bufs=2, space="PSUM") as pssc,
            tc.tile_pool(name="ps_sm", bufs=2, space="PSUM") as pssm,
            tc.tile_pool(name="ps_pv", bufs=2, space="PSUM") as pspv,
            tc.tile_pool(name="ps_vt", bufs=2, space="PSUM") as psvt,
        ):
            q_t = cpool.tile([128, 128], dt.bfloat16, tag="qT")
            id_t = cpool.tile([128, 128], dt.bfloat16, tag="ident")
            ones_t = cpool.tile([128, 1], dt.bfloat16, tag="ones")
            idx_t = cpool.tile([128, idx_w], dt.int16, tag="idxall")
            # first group's idx slice first so its gather starts early
            _w0 = info[1]["halves"][0]["ioff"] if GROUPS > 1 else idx_w
            _w0 = max(1, min(_w0, idx_w))
            nc.sync.dma_start(out=idx_t[:, 0:_w0], in_=idx_d[:, 0:_w0])
            if _w0 < idx_w:
                nc.sync.dma_start(out=idx_t[:, _w0:idx_w], in_=idx_d[:, _w0:idx_w])
            nc.sync.dma_start(out=q_t[:], in_=qT_d[:])
            nc.sync.dma_start(out=id_t[:], in_=ident_d[:])
            nc.vector.memset(ones_t[:], 1.0)

            for g in range(GROUPS):
                gi = info[g]
                nslots = gi["nslots"]
                ncols = 4 * nslots
                if nslots == 0:
                    z = stgp.tile([G, OC], dt.float32, tag="ostg")
                    nc.vector.memset(z[:], 0.0)
                    nc.sync.dma_start(out=o_dram[0:G, OC * g:OC * (g + 1)],
                                      in_=z[:])
                    continue
                # --- gather combined K|V rows for both pool halves ---------
                kvt_tiles = {}
                for h in (0, 1):
                    n = gi["halves"][h]["n"]
                    if n == 0:
                        continue
                    ioff = gi["halves"][h]["ioff"]
                    it = idx_t[:, ioff:ioff + n // 16]
                    src = kv_il[0:HALF, :] if h == 0 else kv_il[HALF:POOL, :]
                    kvt = kvtp.tile([128, 2, n], dt.bfloat16, tag="kvt")
                    nc.gpsimd.dma_gather(
                        out_ap=kvt[:], in_ap=src, idxs_ap=it,
                        num_idxs=n, num_idxs_reg=n, elem_size=256,
                        transpose=True, single_packet=False)
                    kvt_tiles[h] = kvt

                pieces = gi["pieces"]
                npieces = len(pieces)
                first_pid = {j: pj[0] for j, pj in enumerate(gi["req_pieces"])
                             if pj}
                last_pid = {j: pj[-1] for j, pj in enumerate(gi["req_pieces"])
                            if pj}

                sc = pssc.tile([128, ncols], dt.float32, tag="sc")
                phi = php.tile([128, ncols], dt.bfloat16, tag="phi")
                vnat = vnp.tile([128, 128 * nslots], dt.bfloat16, tag="vnat")
                pv = pspv.tile([G, OC], dt.float32, tag="pv")
                sm = pssm.tile([1, 4 * npieces], dt.float32, tag="sm")
                for j in range(RPG):
                    if not gi["req_pieces"][j]:
                        nc.vector.memset(pv[0:G, 128 * j:128 * (j + 1)], 0.0)

                nchunks = (nslots + 7) // 8
                chunk_pieces = [[] for _ in range(nchunks)]
                for p in pieces:
                    chunk_pieces[p[0] // 8].append(p)

                # request j's PV chain is emitted as one sequential run (a
                # PSUM bank allows only one open accumulation group) once the
                # chunk holding its last piece has been exp'd/evacuated
                done_chunk = {j: pieces[last_pid[j]][0] // 8
                              for j in last_pid}

                # start&stop matmuls are atomic groups; the interp's global
                # group-check shadow aliases partition-offset APs across
                # banks, so skip it for them (data semantics are per-tile
                # and unaffected)
                def emit_sums(c):
                    for (s, t0, ln, j, pid) in chunk_pieces[c]:
                        nc.tensor.matmul(sm[0:1, 4 * pid:4 * pid + 4],
                                         ones_t[t0:t0 + ln, 0:1],
                                         phi[t0:t0 + ln, 4 * s:4 * s + 4],
                                         start=True, stop=True)

                def emit_chains(c):
                    for j in range(RPG):
                        if done_chunk.get(j) != c:
                            continue
                        oc = 128 * j
                        for pid in gi["req_pieces"][j]:
                            s, t0, ln, _, _ = pieces[pid]
                            nc.tensor.matmul(
                                pv[0:G, oc:oc + 128],
                                phi[t0:t0 + ln, 4 * s:4 * s + 4],
                                vnat[t0:t0 + ln, 128 * s:128 * (s + 1)],
                                start=(pid == first_pid[j]),
                                stop=(pid == last_pid[j]))

                # chunked pipeline: QK+VT+exp+evac of chunk c, then PV+sums
                # of chunk c-1 (so PE never waits on ACT/DVE of the same
                # chunk); drain tail is just the last chunk's chain.
                for c in range(nchunks):
                    c0 = 8 * c
                    cn = min(8, nslots - c0)
                    # QK piece matmuls
                    for (s, t0, ln, j, pid) in chunk_pieces[c]:
                        h, loc = gi["slot_map"][s]
                        b = RPG * g + (j if j is not None else RPG - 1)
                        kT = kvt_tiles[h][:, 0,
                                          128 * loc + t0:128 * loc + t0 + ln]
                        nc.tensor.matmul(sc[t0:t0 + ln, 4 * s:4 * s + 4], kT,
                                         q_t[:, 4 * b:4 * b + 4],
                                         start=True, stop=True)
                    # exp of this chunk's score columns
                    nc.scalar.activation(phi[:, 4 * c0:4 * (c0 + cn)],
                                         sc[:, 4 * c0:4 * (c0 + cn)],
                                         mybir.ActivationFunctionType.Exp)
                    # V^T -> V transposes into a bf16 PSUM bank
                    vtb = psvt.tile([128, 1024], dt.bfloat16, tag="vtb")
                    for s in range(c0, c0 + cn):
                        h, loc = gi["slot_map"][s]
                        vT = kvt_tiles[h][:, 1, 128 * loc:128 * (loc + 1)]
                        nc.tensor.matmul(
                            vtb[:, 128 * (s - c0):128 * (s - c0 + 1)],
                            vT, id_t[:], is_transpose=True,
                            start=True, stop=True)
                    dst = vnat[:, 128 * c0:128 * (c0 + cn)]
                    if c % 2 == 0:
                        nc.vector.tensor_copy(out=dst, in_=vtb[:, 0:128 * cn])
                    else:
                        nc.scalar.copy(out=dst, in_=vtb[:, 0:128 * cn])
                    if c > 0:
                        emit_sums(c - 1)
                        emit_chains(c - 1)
                emit_sums(nchunks - 1)
                emit_chains(nchunks - 1)

                ostg = stgp.tile([G, OC], dt.float32, tag="ostg")
                sstg = stgp.tile([1, 512], dt.float32, tag="sstg")
                nc.vector.tensor_copy(out=ostg[:], in_=pv[:])
                nc.scalar.copy(out=sstg[0:1, 0:4 * npieces],
                               in_=sm[0:1, 0:4 * npieces])
                nc.sync.dma_start(out=o_dram[0:G, OC * g:OC * (g + 1)],
                                  in_=ostg[:])
                nc.sync.dma_start(out=s_dram[g:g + 1, 0:4 * npieces],
                                  in_=sstg[0:1, 0:4 * npieces])

    nc.compile()
    return nc, info


def prepare(inputs):
    q = np.asarray(inputs["q"], np.float32)
    k = np.asarray(inputs["k"], np.float32)
    v = np.asarray(inputs["v"], np.float32)
    k_buffer = np.asarray(inputs["k_buffer"], np.float32)
    v_buffer = np.asarray(inputs["v_buffer"], np.float32)
    req_to_token = np.asarray(inputs["req_to_token"])
    req_pool_indices = np.asarray(inputs["req_pool_indices"])
    seq_lens = np.asarray(inputs["seq_lens"]).astype(np.int64)
    out_cache_loc = np.asarray(inputs["out_cache_loc"]).astype(np.int64)

    # store_kv_cache scatter (tiny: 32 rows) + per-request token lists
    kb = k_buffer.copy()
    vb = v_buffer.copy()
    kb[out_cache_loc] = k.reshape(B, HKV, D)
    vb[out_cache_loc] = v.reshape(B, HKV, D)
    tok = req_to_token[req_pool_indices]

    # one unreferenced pool row per half as the zero pad target
    used = np.zeros(POOL, bool)
    for b in range(B):
        used[tok[b, :seq_lens[b]]] = True
    free_lo = np.flatnonzero(~used[:HALF])
    free_hi = np.flatnonzero(~used[HALF:]) + HALF
    assert len(free_lo) and len(free_hi), "no free pad row in a pool half"
    z_lo, z_hi = int(free_lo[0]), int(free_hi[0])

    # second-smallest group first (fast pipeline fill), smallest last (short
    # drain tail), the rest biggest-first in between
    asc = list(np.argsort(seq_lens, kind="stable"))
    head, tail_, mid = asc[RPG:2 * RPG], asc[:RPG], asc[2 * RPG:][::-1]
    order = np.array(head + mid + tail_, dtype=np.int64)

    meta = []
    idx_blocks = []
    npad = np.zeros(B, np.int64)  # per processing-position pad token count
    for g in range(GROUPS):
        halves_secs = []
        for h in (0, 1):
            zrow = z_lo if h == 0 else z_hi - HALF
            tls, secs = [], []
            for j in range(RPG):
                b = int(order[RPG * g + j])
                t = tok[b, :seq_lens[b]].astype(np.int64)
                tl = t[t < HALF] if h == 0 else t[t >= HALF] - HALF
                tls.append(tl)
                secs.append(len(tl))
            pads, n = _sec_pads(secs)
            full = np.full(n, zrow, np.int64)
            pos = 0
            for j in range(RPG):
                full[pos:pos + secs[j]] = tls[j]
                npad[RPG * g + j] += pads[j] - secs[j]
                pos += pads[j]
            halves_secs.append(tuple(secs))
            if len(full):
                # [16, n/16] wrap, replicated into all 8 GPSIMD-core stripes
                idx_blocks.append(
                    np.tile(full.astype(np.int16).reshape(-1, 16).T, (8, 1)))
        meta.append(tuple(halves_secs))
    meta = tuple(meta)
    if idx_blocks:
        idx_all = np.ascontiguousarray(np.concatenate(idx_blocks, axis=1))
    else:
        idx_all = np.zeros((128, 1), np.int16)

    if meta not in _prog_cache:
        _prog_cache[meta] = _build_program(meta)
    nc, info = _prog_cache[meta]

    ident = np.eye(128, dtype=BF16)
    in_maps = []
    for c in range(NCORES):
        kh = kb[:, c, :].astype(BF16)
        vh = vb[:, c, :].astype(BF16)
        kv = np.concatenate([kh, vh], axis=1)
        kv[z_lo] = 0
        kv[z_hi] = 0
        qc = (q.reshape(B, HKV, G, D)[order, c] * SCALE).reshape(B * G, D)
        qT = np.ascontiguousarray(qc.T).astype(BF16)
        im = {
            "kv_il": np.ascontiguousarray(kv),
            "qT": qT,
            "ident": ident,
            "idx_all": idx_all,
        }
        in_maps.append(im)
    return nc, info, in_maps, order, npad


def postprocess(results, info, order, npad, cores=None):
    OC = RPG * D
    out = np.zeros((B, HQ, D), np.float32)
    for c in (cores if cores is not None else range(NCORES)):
        o_un = results[c]["o_un"]
        sums = results[c]["sums"]
        for g in range(GROUPS):
            gi = info[g]
            for j in range(RPG):
                pos = RPG * g + j
                b = int(order[pos])
                stot = np.zeros(G, np.float64)
                for pid in gi["req_pieces"][j]:
                    stot += sums[g, 4 * pid:4 * pid + 4].astype(np.float64)
                stot -= npad[pos]  # pad tokens contribute exp(0)=1 each
                ov = o_un[:, OC * g + 128 * j:OC * g + 128 * (j + 1)]
                with np.errstate(divide="ignore", invalid="ignore"):
                    out[b, c * G:(c + 1) * G, :] = ov / stot[:, None]
    return out.reshape(B, HQ * D).astype(np.float32)


def kernel(**inputs):
    global LAST_RESULT
    nc, info, in_maps, order, npad = prepare(inputs)
    res = run_bass_kernel_spmd(nc, in_maps, core_ids=list(range(NCORES)),
                               trace=False)
    LAST_RESULT = res
    return postprocess(res.results, info, order, npad)
